# revision 8
# baseline (speedup 1.0000x reference)
"""MoE expert FFN (E=8, C=2048, D=1024, F=4096) on 8 TRN2 NeuronCores.

Expert-parallel: one expert per core. Per core computes
    out[e] = gelu_tanh(x[e] @ w1[e] + b1[e]) @ w2[e]        (+ b2 on host)
as two matmul phases through the PE array in fp32r (full-rate fp32):

  mm1: hT[F, C] = (w1 stationary).T @ (xT moving), gelu+b1 fused in the
       PSUM->SBUF eviction on the ACT engine, hT spilled to DRAM in
       [cm, fk, 128, 128] blocks.
  mm2: out[C, D] = (hT stationary).T @ (w2 moving), w2 SBUF-resident.
"""

import numpy as np

import concourse.bass as bass
import concourse.mybir as mybir
import concourse.tile as tile
from concourse import bacc
from concourse.bass_utils import run_bass_kernel_spmd

E, C, D, F = 8, 2048, 1024, 4096
P = 128
KD = D // P  # 8   k-tiles in mm1 contraction
MF = F // P  # 32  F row-blocks (mm1 output partitions / mm2 contraction)
CN = C // 512  # 4   token n-tiles in mm1
CM = C // P  # 16  token row-blocks in mm2
DN = D // 512  # 2   d_model n-tiles in mm2

F32 = mybir.dt.float32
F32R = mybir.dt.float32r
GELU = mybir.ActivationFunctionType.Gelu_apprx_tanh

_CACHE = {}


def _build():
    nc = bacc.Bacc("TRN2", target_bir_lowering=False, debug=False, num_devices=E)

    xT_d = nc.dram_tensor("xT", [KD, P, C], F32R, kind="ExternalInput").ap()
    w1_d = nc.dram_tensor("w1b", [MF, KD, P, P], F32R, kind="ExternalInput").ap()
    b1_d = nc.dram_tensor("b1t", [P, MF], F32, kind="ExternalInput").ap()
    w2_d = nc.dram_tensor("w2b", [MF, DN, P, 512], F32R, kind="ExternalInput").ap()
    hT_d = nc.dram_tensor("hT", [CM, MF, P, P], F32R).ap()
    out_d = nc.dram_tensor("out", [C, D], F32, kind="ExternalOutput").ap()

    with tile.TileContext(nc) as tc:
        # ---------------- mm1: hT = gelu(w1.T @ xT + b1) ----------------
        with (
            tc.tile_pool(name="xt", bufs=1) as xt_pool,
            tc.tile_pool(name="w1", bufs=3) as w1_pool,
            tc.tile_pool(name="b1", bufs=1) as b1_pool,
            tc.tile_pool(name="ev1", bufs=4) as ev1_pool,
            tc.tile_pool(name="ps1", bufs=8, space="PSUM") as ps1_pool,
        ):
            b1t = b1_pool.tile([P, MF], F32)
            nc.sync.dma_start(b1t[:], b1_d[:])

            xt = xt_pool.tile([P, KD, C], F32R)
            for k in range(KD):
                # split each 1MB k-slice in 4 to spread the initial fill
                # across DMA queues
                for q in range(4):
                    s = bass.ds(q * 512, 512)
                    nc.sync.dma_start(xt[:, k, s], xT_d[k, :, s])

            for mf in range(MF):
                w1t = w1_pool.tile([P, KD, P], F32R, tag="w1")
                for k in range(KD):
                    nc.sync.dma_start(w1t[:, k, :], w1_d[mf, k])
                for cn in range(CN):
                    ps = ps1_pool.tile([P, 512], F32, tag="ps1")
                    for k in range(KD):
                        nc.tensor.matmul(
                            ps[:],
                            w1t[:, k, :],
                            xt[:, k, bass.ds(cn * 512, 512)],
                            start=(k == 0),
                            stop=(k == KD - 1),
                        )
                    ev = ev1_pool.tile([P, 512], F32R, tag="ev1")
                    nc.scalar.activation(
                        ev[:], ps[:], GELU, bias=b1t[:, mf : mf + 1]
                    )
                    for j in range(4):
                        nc.sync.dma_start(
                            hT_d[cn * 4 + j, mf], ev[:, bass.ds(j * P, P)]
                        )

        # ---------------- mm2: out = hT.T @ w2 ----------------
        with (
            tc.tile_pool(name="w2", bufs=1) as w2_pool,
            tc.tile_pool(name="ht", bufs=2) as ht_pool,
            tc.tile_pool(name="ev2", bufs=4) as ev2_pool,
            tc.tile_pool(name="ps2", bufs=8, space="PSUM") as ps2_pool,
        ):
            w2t = w2_pool.tile([P, MF, DN, 512], F32R)
            for fk in range(MF):
                for dn in range(DN):
                    nc.sync.dma_start(w2t[:, fk, dn, :], w2_d[fk, dn])

            for cm in range(CM):
                ht = ht_pool.tile([P, MF, P], F32R, tag="ht")
                for fk in range(MF):
                    nc.sync.dma_start(ht[:, fk, :], hT_d[cm, fk])
                for dn in range(DN):
                    ps = ps2_pool.tile([P, 512], F32, tag="ps2")
                    for fk in range(MF):
                        nc.tensor.matmul(
                            ps[:],
                            ht[:, fk, :],
                            w2t[:, fk, dn, :],
                            start=(fk == 0),
                            stop=(fk == MF - 1),
                        )
                    ev = ev2_pool.tile([P, 512], F32, tag="ev2")
                    nc.vector.tensor_copy(ev[:], ps[:])
                    nc.sync.dma_start(
                        out_d[
                            cm * P : (cm + 1) * P, dn * 512 : (dn + 1) * 512
                        ],
                        ev[:],
                    )

    nc.compile()
    return nc


def _get_nc():
    if "nc" not in _CACHE:
        _CACHE["nc"] = _build()
    return _CACHE["nc"]


def _in_map(x_e, w1_e, b1_e, w2_e):
    xT = np.ascontiguousarray(x_e.T).reshape(KD, P, C)
    w1b = np.ascontiguousarray(
        w1_e.reshape(KD, P, MF, P).transpose(2, 0, 1, 3)
    )
    b1t = np.ascontiguousarray(b1_e.reshape(MF, P).T)
    w2b = np.ascontiguousarray(
        w2_e.reshape(MF, P, DN, 512).transpose(0, 2, 1, 3)
    )
    return {"xT": xT, "w1b": w1b, "b1t": b1t, "w2b": w2b}


def kernel(inputs, w1, b1, w2, b2, _trace=False):
    nc = _get_nc()
    x = np.asarray(inputs, dtype=np.float32).reshape(E, C, D)
    in_maps = [
        _in_map(
            x[e],
            np.asarray(w1[e], dtype=np.float32),
            np.asarray(b1[e], dtype=np.float32),
            np.asarray(w2[e], dtype=np.float32),
        )
        for e in range(E)
    ]
    res = run_bass_kernel_spmd(nc, in_maps, list(range(E)), trace=_trace)
    out = np.stack([res.results[e]["out"] for e in range(E)])[None]
    out = out + np.asarray(b2, dtype=np.float32)[None]
    if _trace:
        _CACHE["last_results"] = res
    return out.astype(np.float32)


# revision 9
# speedup vs baseline: 1.0376x; 1.0376x over previous
"""Fully-fused fp16 MoE expert FFN (E=8, C=2048, D=1024, F=4096), 8 TRN2 cores.

One expert per core. w1 AND w2 are fully SBUF-resident in fp16
(64KB/partition each). Per 512-token chunk: mm1 (32 psum groups, gelu+b1
fused in ACT eviction) -> hT chunk in SBUF -> mm2 (8 psum groups, K=4096
accumulation) -> out. No DRAM intermediates at all; total HBM traffic
28MB/core. PE sees one continuous stream of 2048 N=512 matmuls at
1 cycle/row (fp16).
"""

import numpy as np

import concourse.bass as bass
import concourse.mybir as mybir
import concourse.tile as tile
from concourse import bacc
from concourse.bass_utils import run_bass_kernel_spmd

E, C, D, F = 8, 2048, 1024, 4096
P = 128
KD = D // P  # 8
MF = F // P  # 32
CN = C // 512  # 4 chunks of 512 tokens
CJ = 4  # 128-token subblocks per chunk
DN = D // 512  # 2

F32 = mybir.dt.float32
F16 = mybir.dt.float16
GELU = mybir.ActivationFunctionType.Gelu_apprx_tanh

_CACHE = {}


def _build():
    nc = bacc.Bacc("TRN2", target_bir_lowering=False, debug=False, num_devices=E)

    xT_d = nc.dram_tensor("xT", [KD, P, C], F16, kind="ExternalInput").ap()
    w1_d = nc.dram_tensor("w1r", [KD, P, F], F16, kind="ExternalInput").ap()
    b1_d = nc.dram_tensor("b1t", [P, MF], F32, kind="ExternalInput").ap()
    w2_d = nc.dram_tensor("w2r", [MF, P, D], F16, kind="ExternalInput").ap()
    out_d = nc.dram_tensor("out", [C, D], F32, kind="ExternalOutput").ap()

    with tile.TileContext(nc) as tc:
        with (
            tc.tile_pool(name="w1f", bufs=1) as w1_pool,
            tc.tile_pool(name="w2f", bufs=1) as w2_pool,
            tc.tile_pool(name="b1", bufs=1) as b1_pool,
            tc.tile_pool(name="xt", bufs=2) as xt_pool,
            tc.tile_pool(name="ht", bufs=1) as ht_pool,
            tc.tile_pool(name="ev", bufs=4) as ev_pool,
            tc.tile_pool(name="ps1", bufs=4, space="PSUM") as ps1_pool,
            tc.tile_pool(name="ps2", bufs=4, space="PSUM") as ps2_pool,
        ):
            b1t = b1_pool.tile([P, MF], F32)
            nc.sync.dma_start(b1t[:], b1_d[:])

            def load_xt(cn):
                t = xt_pool.tile([P, KD, 512], F16, tag="xt")
                for k in range(KD):
                    nc.sync.dma_start(
                        t[:, k, :], xT_d[k, :, cn * 512 : (cn + 1) * 512]
                    )
                return t

            # chunk-0 activations first so the PE can start immediately;
            # then w1 in F-column-major pieces (psum group j needs column
            # block j for all k), then w2 ordered by dn-half (mm2 group
            # (cj, dn) reads the dn half of every fk row).
            xt0 = load_xt(0)

            w1f = w1_pool.tile([P, KD, F], F16)
            for jj in range(F // 512):
                for k in range(KD):
                    nc.sync.dma_start(
                        w1f[:, k, bass.ds(jj * 512, 512)],
                        w1_d[k, :, jj * 512 : (jj + 1) * 512],
                    )
            w2f = w2_pool.tile([P, MF, D], F16)
            for dn in range(DN):
                for j in range(MF):
                    nc.sync.dma_start(
                        w2f[:, j, bass.ds(dn * 512, 512)],
                        w2_d[j, :, dn * 512 : (dn + 1) * 512],
                    )

            for cn in range(CN):
                xt = xt0 if cn == 0 else load_xt(cn)
                ht = ht_pool.tile([P, MF, 512], F16, tag="ht")
                for j in range(MF):
                    ps = ps1_pool.tile([P, 512], F32, tag="ps1")
                    for k in range(KD):
                        nc.tensor.matmul(
                            ps[:],
                            w1f[:, k, bass.ds(j * P, P)],
                            xt[:, k, :],
                            start=(k == 0),
                            stop=(k == KD - 1),
                        )
                    nc.scalar.activation(
                        ht[:, j, :], ps[:], GELU, bias=b1t[:, j : j + 1]
                    )
                for cj in range(CJ):
                    row = cn * 512 + cj * P
                    for dn in range(DN):
                        ps = ps2_pool.tile([P, 512], F32, tag="ps2")
                        for j in range(MF):
                            nc.tensor.matmul(
                                ps[:],
                                ht[:, j, bass.ds(cj * P, P)],
                                w2f[:, j, bass.ds(dn * 512, 512)],
                                start=(j == 0),
                                stop=(j == MF - 1),
                            )
                        ev = ev_pool.tile([P, 512], F32, tag="ev")
                        nc.vector.tensor_copy(ev[:], ps[:])
                        nc.sync.dma_start(
                            out_d[row : row + P, dn * 512 : (dn + 1) * 512],
                            ev[:],
                        )

    nc.compile()
    return nc


def _get_nc():
    if "nc" not in _CACHE:
        _CACHE["nc"] = _build()
    return _CACHE["nc"]


def _in_map(x_e, w1_e, b1_e, w2_e):
    xT = np.ascontiguousarray(x_e.T).astype(np.float16).reshape(KD, P, C)
    w1r = w1_e.astype(np.float16).reshape(KD, P, F)
    b1t = np.ascontiguousarray(b1_e.reshape(MF, P).T)
    w2r = w2_e.astype(np.float16).reshape(MF, P, D)
    return {"xT": xT, "w1r": w1r, "b1t": b1t, "w2r": w2r}


def kernel(inputs, w1, b1, w2, b2, _trace=False):
    nc = _get_nc()
    x = np.asarray(inputs, dtype=np.float32).reshape(E, C, D)
    in_maps = [
        _in_map(
            x[e],
            np.asarray(w1[e], dtype=np.float32),
            np.asarray(b1[e], dtype=np.float32),
            np.asarray(w2[e], dtype=np.float32),
        )
        for e in range(E)
    ]
    res = run_bass_kernel_spmd(nc, in_maps, list(range(E)), trace=_trace)
    out = np.stack([res.results[e]["out"] for e in range(E)])[None]
    out = out + np.asarray(b2, dtype=np.float32)[None]
    if _trace:
        _CACHE["last_results"] = res
    return out.astype(np.float32)
